# revision 1
# baseline (speedup 1.0000x reference)
"""Trainium2 Bass kernel for the DDF (dynamic-filter + ECA + BN) module.

Distribution: data-parallel over batch B=8 across 8 NeuronCores (one image
per core).  All parameters replicated.  BN batch stats are all-reduced
across cores (sync-BN semantics, matching the reference).

Per-core layout: channels on partitions (2 channel-tiles of 128), pixels on
the free dimension.  The per-pixel filter generator (1x1 conv C -> C*9) is
permuted on the host to o' = k*256 + c so that each PE output m-tile is one
(tap k, channel-tile) pair.  The 3x3 shifted windows come from three flat
guard-row buffers prepared on the host (center / left-shifted /
right-shifted, each [64 zeros][x flat][64 zeros]), which makes every tap
window a contiguous 1024-element slice — keeping the DVE's bf16 2x packing
mode legal and the matmul rhs streams contiguous.  The projection matmul
folds the 0.5 fusion factor into its weights; b_proj is dropped entirely
(batch-norm cancels any per-channel constant).
"""

import os

import numpy as np
import ml_dtypes

import concourse.bass as bass
import concourse.mybir as mybir
import concourse.tile as tile
from concourse import bacc
from concourse.bass_utils import run_bass_kernel_spmd

B, C, H, W = 8, 256, 64, 64
KS = 3
HW = H * W                    # 4096
GUARD = W                     # zero guard rows (one image row) at each end
XBUF = GUARD + HW + GUARD     # 4224
NCORES = 8
CT = 2                        # channel tiles of 128
MT1 = KS * KS * CT            # 18 mm1 output m-tiles
BN_EPS = 1e-5
F32 = mybir.dt.float32
BF16 = mybir.dt.bfloat16
ROWS_PER_CHUNK = 16
NCHUNKS = H // ROWS_PER_CHUNK  # 4
CHUNK = ROWS_PER_CHUNK * W     # 1024 pixels per chunk per channel-tile
NH = CHUNK // 512              # 512-px matmul groups per chunk

AF = mybir.ActivationFunctionType
ALU = mybir.AluOpType


def _emit(tc):
    nc = tc.nc

    # x window buffers: [dj] 0=left-shifted, 1=center, 2=right-shifted
    xb = [
        nc.declare_dram_parameter(f"xb{d}", [CT, 128, XBUF], BF16, isOutput=False)
        for d in range(KS)
    ]
    wf = nc.declare_dram_parameter("wf", [CT, 128, MT1 * 128], BF16, isOutput=False)
    bfp = nc.declare_dram_parameter("bfp", [128, MT1], F32, isOutput=False)
    wp = nc.declare_dram_parameter("wp", [CT, 128, C], BF16, isOutput=False)
    weca = nc.declare_dram_parameter("weca", [1, 3], F32, isOutput=False)
    gam = nc.declare_dram_parameter("gam", [128, CT], F32, isOutput=False)
    bet = nc.declare_dram_parameter("bet", [128, CT], F32, isOutput=False)
    yout = nc.declare_dram_parameter("y", [CT, 128, HW], F32, isOutput=True)

    with (
        tc.tile_pool(name="consts", bufs=1) as consts,
        tc.tile_pool(name="fps", bufs=3, space="PSUM") as fps,
        tc.tile_pool(name="yps", bufs=2, space="PSUM") as yps,
        tc.tile_pool(name="fsb", bufs=6) as fsb_pool,
        tc.tile_pool(name="prod", bufs=2) as prod_pool,
        tc.tile_pool(name="fused", bufs=2) as fused_pool,
        tc.tile_pool(name="dram", bufs=1, space="DRAM") as dram,
    ):
        # ---- resident tensors -------------------------------------------
        wf_sb = [consts.tile([128, MT1 * 128], BF16, tag=f"wf{kt}", name=f"wf{kt}")
                 for kt in range(CT)]
        wp_sb = [consts.tile([128, C], BF16, tag=f"wp{kt}", name=f"wp{kt}")
                 for kt in range(CT)]
        bfp_sb = consts.tile([128, MT1], F32, tag="bfp", name="bfp")
        gam_sb = consts.tile([128, CT], F32, tag="gam", name="gam")
        bet_sb = consts.tile([128, CT], F32, tag="bet", name="bet")
        wecab = consts.tile([128, 3], F32, tag="wecab", name="wecab")
        xb_sb = [
            [consts.tile([128, XBUF], BF16, tag=f"xb{d}_{ct}", name=f"xb{d}_{ct}")
             for ct in range(CT)]
            for d in range(KS)
        ]
        y_sb = [consts.tile([128, HW], F32, tag=f"ysb{mt}", name=f"ysb{mt}")
                for mt in range(CT)]
        stats_sb = [
            consts.tile([128, NCHUNKS * NH, 6], F32, tag=f"st{mt}", name=f"st{mt}")
            for mt in range(CT)
        ]

        # Input DMA order = what the first matmuls need first: mm1 weight
        # slice 0, then chunk-0 center pixels, bias, chunk-0 side buffers,
        # then the rest.
        WFS = MT1 * 128 // 3
        cuts = [0, GUARD + CHUNK + GUARD]
        for ci in range(1, NCHUNKS):
            cuts.append(GUARD + (ci + 1) * CHUNK + GUARD)
        cuts[-1] = XBUF

        def dma_piece(d, ct, ci):
            nc.sync.dma_start(
                out=xb_sb[d][ct][:, cuts[ci] : cuts[ci + 1]],
                in_=xb[d][ct, :, cuts[ci] : cuts[ci + 1]],
            )

        for kt in range(CT):
            nc.sync.dma_start(
                out=wf_sb[kt][:, 0:WFS], in_=wf[kt, :, 0:WFS]
            )
        for ct in range(CT):
            dma_piece(1, ct, 0)
        nc.sync.dma_start(out=bfp_sb[:], in_=bfp[:, :])
        for ct in range(CT):
            dma_piece(0, ct, 0)
            dma_piece(2, ct, 0)
        for s in range(1, 3):
            for kt in range(CT):
                nc.sync.dma_start(
                    out=wf_sb[kt][:, s * WFS : (s + 1) * WFS],
                    in_=wf[kt, :, s * WFS : (s + 1) * WFS],
                )
        for ci in range(1, NCHUNKS):
            for d in (1, 0, 2):
                for ct in range(CT):
                    dma_piece(d, ct, ci)
        for kt in range(CT):
            nc.sync.dma_start(out=wp_sb[kt][:], in_=wp[kt])
        nc.sync.dma_start(out=gam_sb[:], in_=gam[:, :])
        nc.sync.dma_start(out=bet_sb[:], in_=bet[:, :])
        nc.sync.dma_start(out=wecab[:], in_=weca[0:1, :].to_broadcast([128, 3]))

        def win(d, ct, row0, npix=CHUNK):
            """Contiguous window slice: npix pixels starting at image row row0
            of buffer d (row0 may be -1..64; guards supply zeros)."""
            off = GUARD + row0 * W
            return xb_sb[d][ct][:, off : off + npix]

        # ---- ECA channel attention --------------------------------------
        # pooled[c] = sum_p x[c, p] (the 1/HW is pre-folded into w_eca),
        # accumulated per DMA piece so it tracks the input as it arrives.
        poolp = consts.tile([128, CT, NCHUNKS], F32, tag="poolp", name="poolp")
        pool2 = consts.tile([128, CT], F32, tag="pool2", name="pool2")
        for ct in range(CT):
            for ci in range(NCHUNKS):
                nc.vector.tensor_reduce(
                    out=poolp[:, ct, ci : ci + 1],
                    in_=xb_sb[1][ct][:, GUARD + ci * CHUNK : GUARD + (ci + 1) * CHUNK],
                    axis=mybir.AxisListType.X,
                    op=ALU.add,
                )
            nc.vector.tensor_reduce(
                out=pool2[:, ct : ct + 1],
                in_=poolp[:, ct, :],
                axis=mybir.AxisListType.X,
                op=ALU.add,
            )
        # channel-shifted copies (zero pad at the ends); shifts cross the
        # two channel-tiles via tiny partition-offset DMAs.
        shd = consts.tile([128, CT], F32, tag="shd", name="shd")  # pooled[c-1]
        shu = consts.tile([128, CT], F32, tag="shu", name="shu")  # pooled[c+1]
        nc.vector.memset(shd[:], 0.0)
        nc.vector.memset(shu[:], 0.0)
        for ct in range(CT):
            nc.gpsimd.dma_start(
                out=shd[1:128, ct : ct + 1], in_=pool2[0:127, ct : ct + 1]
            )
            nc.gpsimd.dma_start(
                out=shu[0:127, ct : ct + 1], in_=pool2[1:128, ct : ct + 1]
            )
        nc.gpsimd.dma_start(out=shd[0:1, 1:2], in_=pool2[127:128, 0:1])
        nc.gpsimd.dma_start(out=shu[127:128, 0:1], in_=pool2[0:1, 1:2])

        eca1 = consts.tile([128, CT], F32, tag="eca1", name="eca1")
        eca2 = consts.tile([128, CT], F32, tag="eca2", name="eca2")
        attn = consts.tile([128, CT], F32, tag="attn", name="attn")
        nc.vector.tensor_scalar(
            out=eca1, in0=shd[:], scalar1=wecab[:, 0:1], scalar2=None, op0=ALU.mult
        )
        nc.vector.scalar_tensor_tensor(
            out=eca2, in0=pool2[:], scalar=wecab[:, 1:2], in1=eca1[:],
            op0=ALU.mult, op1=ALU.add,
        )
        nc.vector.scalar_tensor_tensor(
            out=eca1, in0=shu[:], scalar=wecab[:, 2:3], in1=eca2[:],
            op0=ALU.mult, op1=ALU.add,
        )
        # attn = sigmoid(eca) = 1 / (1 + exp(-eca))
        nc.scalar.activation(out=eca2[:], in_=eca1[:], func=AF.Exp, scale=-1.0)
        nc.vector.tensor_scalar(
            out=attn, in0=eca2[:], scalar1=1.0, scalar2=None, op0=ALU.add
        )
        nc.vector.reciprocal(out=attn[:], in_=attn[:])

        # ---- main loop over row chunks ----------------------------------
        for ci in range(NCHUNKS):
            r0 = ci * ROWS_PER_CHUNK
            prods = []
            for k in range(KS * KS):
                di, dj = divmod(k, KS)
                pr = prod_pool.tile([128, CT * CHUNK], BF16, tag=f"pr{k}",
                                    name=f"pr{k}")
                for ct in range(CT):
                    mt = k * CT + ct
                    fp = fps.tile([128, CHUNK], F32, tag="fp", name="fp")
                    for kt in range(CT):
                        lhsT = wf_sb[kt][:, mt * 128 : (mt + 1) * 128]
                        for nh in range(NH):
                            rhs = win(1, kt, r0 + nh * 8, 512)
                            nc.tensor.matmul(
                                fp[:, nh * 512 : (nh + 1) * 512],
                                lhsT,
                                rhs,
                                start=(kt == 0),
                                stop=(kt == CT - 1),
                            )
                    # evict + bias, cast to bf16
                    fsb = fsb_pool.tile([128, CHUNK], BF16, tag="fsb", name="fsb")
                    nc.scalar.activation(
                        out=fsb[:], in_=fp[:], func=AF.Identity,
                        bias=bfp_sb[:, mt : mt + 1], scale=1.0,
                    )
                    # tap product against the shifted window
                    nc.vector.tensor_tensor(
                        out=pr[:, ct * CHUNK : (ct + 1) * CHUNK],
                        in0=fsb[:],
                        in1=win(dj, ct, r0 + di - 1),
                        op=ALU.mult,
                    )
                prods.append(pr)

            # channel-attention feature
            cfb = prod_pool.tile([128, CT * CHUNK], BF16, tag="cf", name="cf")
            for ct in range(CT):
                nc.vector.tensor_scalar(
                    out=cfb[:, ct * CHUNK : (ct + 1) * CHUNK],
                    in0=win(1, ct, r0),
                    scalar1=attn[:, ct : ct + 1],
                    scalar2=None,
                    op0=ALU.mult,
                )

            # in-place pairwise add tree: fused = p0..p6 + cf (taps 7 and 8
            # join the projection matmul's contraction instead)
            nc.vector.tensor_add(prods[0][:], prods[0][:], prods[1][:])
            nc.vector.tensor_add(prods[2][:], prods[2][:], prods[3][:])
            nc.vector.tensor_add(prods[4][:], prods[4][:], prods[5][:])
            nc.vector.tensor_add(prods[6][:], prods[6][:], cfb[:])
            nc.vector.tensor_add(prods[0][:], prods[0][:], prods[2][:])
            nc.vector.tensor_add(prods[4][:], prods[4][:], prods[6][:])
            fused_t = fused_pool.tile([128, CT * CHUNK], BF16, tag="fused",
                                      name="fused")
            nc.vector.tensor_add(fused_t[:], prods[0][:], prods[4][:])

            # projection matmul + y eviction + local BN stats
            mm2_srcs = [fused_t, prods[7], prods[8]]
            for mt2 in range(CT):
                ypt = [yps.tile([128, 512], F32, tag="yp", name="yp")
                       for _ in range(NH)]
                nsrc = len(mm2_srcs)
                for si, srct in enumerate(mm2_srcs):
                    for kt in range(CT):
                        lhsT2 = wp_sb[kt][:, mt2 * 128 : (mt2 + 1) * 128]
                        for nh in range(NH):
                            nc.tensor.matmul(
                                ypt[nh][:],
                                lhsT2,
                                srct[:, kt * CHUNK + nh * 512 : kt * CHUNK + (nh + 1) * 512],
                                start=(si == 0 and kt == 0),
                                stop=(si == nsrc - 1 and kt == CT - 1),
                            )
                for nh in range(NH):
                    dst = y_sb[mt2][:, r0 * W + nh * 512 : r0 * W + (nh + 1) * 512]
                    nc.scalar.activation(out=dst, in_=ypt[nh][:], func=AF.Copy)
                    nc.vector.bn_stats(
                        out=stats_sb[mt2][:, ci * NH + nh, :], in_=dst
                    )

        # ---- global BN stats via all-reduce -----------------------------
        ps = consts.tile([128, 2 * CT], F32, tag="ps", name="ps")
        for mt2 in range(CT):
            mv = consts.tile([128, 2], F32, tag=f"mv{mt2}", name=f"mv{mt2}")
            nc.vector.bn_aggr(out=mv[:], in_=stats_sb[mt2][:])
            mean = mv[:, 0:1]
            var = mv[:, 1:2]
            nc.vector.tensor_scalar(
                out=ps[:, 2 * mt2 : 2 * mt2 + 1], in0=mean, scalar1=float(HW),
                scalar2=None, op0=ALU.mult,
            )
            # sumsq = (var + mean^2) * HW
            nc.vector.scalar_tensor_tensor(
                out=ps[:, 2 * mt2 + 1 : 2 * mt2 + 2], in0=mean, scalar=mean,
                in1=var, op0=ALU.mult, op1=ALU.add,
            )
            nc.vector.tensor_scalar(
                out=ps[:, 2 * mt2 + 1 : 2 * mt2 + 2],
                in0=ps[:, 2 * mt2 + 1 : 2 * mt2 + 2],
                scalar1=float(HW), scalar2=None, op0=ALU.mult,
            )

        ps_b = dram.tile([128, 2 * CT], F32, tag="psb", name="psb")
        gs_b = dram.tile([128, 2 * CT], F32, tag="gsb", name="gsb")
        nc.sync.dma_start(out=ps_b[:], in_=ps[:])
        nc.gpsimd.collective_compute(
            "AllReduce",
            ALU.add,
            replica_groups=[list(range(NCORES))],
            ins=[ps_b[:].opt()],
            outs=[gs_b[:].opt()],
        )
        gs = consts.tile([128, 2 * CT], F32, tag="gs", name="gs")
        nc.sync.dma_start(out=gs[:], in_=gs_b[:])

        # ---- normalize and write out ------------------------------------
        minv = 1.0 / float(B * HW)
        NSL = 4  # normalize/store slices per channel-tile
        SL = HW // NSL
        mg = consts.tile([128, CT], F32, tag="mg", name="mg")
        vg = consts.tile([128, CT], F32, tag="vg", name="vg")
        rr = consts.tile([128, CT], F32, tag="rr", name="rr")
        tt = consts.tile([128, CT], F32, tag="tt", name="tt")
        ac = consts.tile([128, CT], F32, tag="ac", name="ac")
        bc = consts.tile([128, CT], F32, tag="bc", name="bc")
        # mean and E[y^2] (gs columns are [s0, q0, s1, q1])
        gsv = gs.rearrange("p (m two) -> p m two", two=2)
        nc.vector.tensor_scalar(
            out=mg[:], in0=gsv[:, :, 0], scalar1=minv, scalar2=None, op0=ALU.mult
        )
        nc.vector.tensor_scalar(
            out=vg[:], in0=gsv[:, :, 1], scalar1=minv, scalar2=None, op0=ALU.mult
        )
        # vg = E[y^2] - mean^2 + eps  (via -(mean^2 - E[y^2]) + eps)
        nc.vector.tensor_tensor(out=tt[:], in0=mg[:], in1=mg[:], op=ALU.mult)
        nc.vector.tensor_tensor(out=vg[:], in0=tt[:], in1=vg[:], op=ALU.subtract)
        nc.vector.tensor_scalar(
            out=vg[:], in0=vg[:], scalar1=-1.0, scalar2=BN_EPS,
            op0=ALU.mult, op1=ALU.add,
        )
        # rstd via quake seed + 3 Newton iterations, all on the DVE
        vg_i = vg.bitcast(mybir.dt.int32)
        rr_i = rr.bitcast(mybir.dt.int32)
        nc.vector.tensor_scalar(
            out=rr_i[:], in0=vg_i[:], scalar1=1, scalar2=None,
            op0=ALU.arith_shift_right,
        )
        nc.vector.tensor_scalar(
            out=rr_i[:], in0=rr_i[:], scalar1=-1, scalar2=0x5F3759DF,
            op0=ALU.mult, op1=ALU.add,
        )
        hv = consts.tile([128, CT], F32, tag="hv", name="hv")
        nc.vector.tensor_scalar(
            out=hv[:], in0=vg[:], scalar1=0.5, scalar2=None, op0=ALU.mult
        )
        for _ in range(3):
            nc.vector.tensor_tensor(out=tt[:], in0=rr[:], in1=rr[:], op=ALU.mult)
            nc.vector.tensor_tensor(out=tt[:], in0=tt[:], in1=hv[:], op=ALU.mult)
            nc.vector.tensor_scalar(
                out=tt[:], in0=tt[:], scalar1=-1.0, scalar2=1.5,
                op0=ALU.mult, op1=ALU.add,
            )
            nc.vector.tensor_tensor(out=rr[:], in0=rr[:], in1=tt[:], op=ALU.mult)
        # A = rstd * gamma ; bc = beta - mean * A
        nc.vector.tensor_tensor(out=ac[:], in0=rr[:], in1=gam_sb[:], op=ALU.mult)
        nc.vector.tensor_tensor(out=bc[:], in0=mg[:], in1=ac[:], op=ALU.mult)
        nc.vector.tensor_tensor(out=bc[:], in0=bet_sb[:], in1=bc[:], op=ALU.subtract)
        for mt2 in range(CT):
            for s in range(NSL):
                sl = slice(s * SL, (s + 1) * SL)
                if mt2 == 0:
                    nc.vector.tensor_scalar(
                        out=y_sb[mt2][:, sl], in0=y_sb[mt2][:, sl],
                        scalar1=ac[:, mt2 : mt2 + 1], scalar2=bc[:, mt2 : mt2 + 1],
                        op0=ALU.mult, op1=ALU.add,
                    )
                else:
                    nc.scalar.activation(
                        out=y_sb[mt2][:, sl], in_=y_sb[mt2][:, sl],
                        func=AF.Identity, bias=bc[:, mt2 : mt2 + 1],
                        scale=ac[:, mt2 : mt2 + 1],
                    )
                nc.sync.dma_start(out=yout[mt2, :, sl], in_=y_sb[mt2][:, sl])


_NC = None


def _build_nc(debug=False):
    nc = bacc.Bacc(
        "TRN2", target_bir_lowering=False, debug=debug, num_devices=NCORES
    )
    with tile.TileContext(nc, num_cores=NCORES) as tc:
        _emit(tc)
    nc.compile()
    return nc


def _get_nc():
    global _NC
    if _NC is None:
        _NC = _build_nc()
    return _NC


def _prep_in_maps(x, W_filter, b_filter, w_eca, W_proj, gamma, beta):
    bf = ml_dtypes.bfloat16
    x = np.asarray(x, np.float32)
    W_filter = np.asarray(W_filter, np.float32)
    b_filter = np.asarray(b_filter, np.float32)
    w_eca = np.asarray(w_eca, np.float32)
    W_proj = np.asarray(W_proj, np.float32)
    gamma = np.asarray(gamma, np.float32)
    beta = np.asarray(beta, np.float32)

    # guard-row window buffers: [64 zeros][x shifted by dj-1 cols][64 zeros]
    xbufs = []
    for d in range(KS):
        sh = np.zeros((B, C, H, W), np.float32)
        if d == 0:
            sh[:, :, :, 1:] = x[:, :, :, :-1]
        elif d == 1:
            sh = x
        else:
            sh[:, :, :, :-1] = x[:, :, :, 1:]
        buf = np.zeros((B, C, XBUF), np.float32)
        buf[:, :, GUARD : GUARD + HW] = sh.reshape(B, C, HW)
        xbufs.append(np.ascontiguousarray(buf.reshape(B, CT, 128, XBUF)).astype(bf))

    # permute mm1 weights: o' = k*256 + c  (original o = c*9 + k)
    wperm = W_filter.reshape(C, KS * KS, C).transpose(1, 0, 2).reshape(KS * KS * C, C)
    wf_h = np.ascontiguousarray(wperm.T.reshape(CT, 128, MT1 * 128)).astype(bf)
    bperm = b_filter.reshape(C, KS * KS).T.reshape(KS * KS * C)
    bfp_h = np.ascontiguousarray(bperm.reshape(MT1, 128).T).astype(np.float32)

    wp_h = np.ascontiguousarray((0.5 * W_proj).T.reshape(CT, 128, C)).astype(bf)
    weca_h = (w_eca / float(HW)).reshape(1, 3).astype(np.float32)
    gam_h = np.ascontiguousarray(gamma.reshape(CT, 128).T).astype(np.float32)
    bet_h = np.ascontiguousarray(beta.reshape(CT, 128).T).astype(np.float32)

    in_maps = []
    for i in range(B):
        m = {
            "wf": wf_h,
            "bfp": bfp_h,
            "wp": wp_h,
            "weca": weca_h,
            "gam": gam_h,
            "bet": bet_h,
        }
        for d in range(KS):
            m[f"xb{d}"] = xbufs[d][i]
        in_maps.append(m)
    return in_maps


last_result = None


def kernel(x, W_filter, b_filter, w_eca, W_proj, b_proj, gamma, beta):
    """Full-input, full-output DDF module on 8 NeuronCores."""
    global last_result
    # b_proj is mathematically cancelled by the batch-norm; unused.
    in_maps = _prep_in_maps(x, W_filter, b_filter, w_eca, W_proj, gamma, beta)
    nc = _get_nc()
    trace = bool(int(os.environ.get("DDF_TRACE", "0")))
    res = run_bass_kernel_spmd(nc, in_maps, list(range(NCORES)), trace=trace)
    last_result = res
    out = np.stack(
        [res.results[i]["y"].reshape(C, H, W).astype(np.float32) for i in range(B)]
    )
    return out



# revision 10
# speedup vs baseline: 1.3137x; 1.3137x over previous
"""Trainium2 Bass kernel for the DDF (dynamic-filter + ECA + BN) module.

Distribution: data-parallel over batch B=8 across 8 NeuronCores (one image
per core).  All parameters replicated.  BN batch stats are all-reduced
across cores (sync-BN semantics, matching the reference).

Per-core layout: channels on partitions (2 channel-tiles of 128 on a single
[128, 2, XBUF] buffer), pixels on the free dimension.  Only the CENTER
window buffer is sent from HBM; the column-shifted copies are derived
on-device with SBUF->SBUF DMAs plus strided zero-fills of the wrapped
columns (gpsimd).  The per-pixel filter generator (1x1 conv C -> C*9) is
permuted on the host to o' = k*256 + c so that each PE output m-tile is one
(tap k, channel-tile) pair.  ECA channel attention is folded into a second
copy of the projection weights (W_proj * attn per input channel), so the
channel branch rides the mm2 contraction; taps 7 and 8 join it too.  ECA
pooling uses scalar-engine activations with accum_out; the tiny eca conv
runs on gpsimd so the vector engine never stalls for it.  BN statistics are
taken directly from the mm2 PSUM tiles; the mm2 output is copied to SBUF by
the DMA engines (not compute).  Sums are exchanged with a single 2KB
AllReduce, preceded by a warmup AllReduce at kernel start.

Emission is software-pipelined: mm2 of chunk i is emitted after mm1 of
chunk i+1 so the tensor engine never waits on the vector-engine add tree.
"""

import os

import numpy as np
import ml_dtypes

import concourse.bass as bass
import concourse.mybir as mybir
import concourse.tile as tile
from concourse import bacc
from concourse.bass_utils import run_bass_kernel_spmd

B, C, H, W = 8, 256, 64, 64
KS = 3
HW = H * W                    # 4096
GUARD = W                     # zero guard rows (one image row) at each end
XBUF = GUARD + HW + GUARD     # 4224
NCORES = 8
CT = 2                        # channel tiles of 128
MT1 = KS * KS * CT            # 18 mm1 output m-tiles
BN_EPS = 1e-5
F32 = mybir.dt.float32
BF16 = mybir.dt.bfloat16
ROWS_PER_CHUNK = 16
NCHUNKS = H // ROWS_PER_CHUNK  # 4
CHUNK = ROWS_PER_CHUNK * W     # 1024 pixels per chunk per channel-tile
NH = CHUNK // 512              # 512-px matmul groups per chunk

AF = mybir.ActivationFunctionType
ALU = mybir.AluOpType
RG = [list(range(NCORES))]


def _emit(tc):
    nc = tc.nc

    xbp = nc.declare_dram_parameter("xb", [CT, 128, XBUF], BF16, isOutput=False)
    wf = nc.declare_dram_parameter("wf", [CT, 128, MT1 * 128], BF16, isOutput=False)
    bfp = nc.declare_dram_parameter("bfp", [128, MT1], F32, isOutput=False)
    wp = nc.declare_dram_parameter("wp", [CT, 128, C], BF16, isOutput=False)
    weca = nc.declare_dram_parameter("weca", [1, 3], F32, isOutput=False)
    gam = nc.declare_dram_parameter("gam", [128, CT], F32, isOutput=False)
    bet = nc.declare_dram_parameter("bet", [128, CT], F32, isOutput=False)
    yout = nc.declare_dram_parameter("y", [CT, 128, HW], BF16, isOutput=True)

    with (
        tc.tile_pool(name="consts", bufs=1) as consts,
        tc.tile_pool(name="fps", bufs=3, space="PSUM") as fps,
        tc.tile_pool(name="yps", bufs=2, space="PSUM") as yps,
        tc.tile_pool(name="fsb", bufs=4) as fsb_pool,
        tc.tile_pool(name="prod", bufs=1) as prod_pool,
        tc.tile_pool(name="stage", bufs=4) as stage_pool,
        tc.tile_pool(name="dram", bufs=1, space="DRAM") as dram,
    ):
        # ---- resident tensors -------------------------------------------
        wf_sb = [consts.tile([128, MT1 * 128], BF16, tag=f"wf{kt}", name=f"wf{kt}")
                 for kt in range(CT)]
        wp_sb = [consts.tile([128, C], BF16, tag=f"wp{kt}", name=f"wp{kt}")
                 for kt in range(CT)]
        weff = [consts.tile([128, C], BF16, tag=f"we{kt}", name=f"we{kt}")
                for kt in range(CT)]
        bfp_sb = consts.tile([128, MT1], F32, tag="bfp", name="bfp")
        gam_sb = consts.tile([128, CT], F32, tag="gam", name="gam")
        bet_sb = consts.tile([128, CT], F32, tag="bet", name="bet")
        wecab = consts.tile([128, 3], F32, tag="wecab", name="wecab")
        # window buffers, [dj] 0=left-shifted, 1=center, 2=right-shifted
        xb3 = [consts.tile([128, CT, XBUF], BF16, tag=f"xb{d}", name=f"xb{d}")
               for d in range(KS)]
        y_sb = [consts.tile([128, HW], F32, tag=f"ysb{mt}", name=f"ysb{mt}")
                for mt in range(CT)]
        stats_sb = [
            consts.tile([128, NCHUNKS * NH, 6], F32, tag=f"st{mt}", name=f"st{mt}")
            for mt in range(CT)
        ]
        pscr = consts.tile([128, CHUNK], BF16, tag="pscr", name="pscr")
        pacc = consts.tile([128, CT, NCHUNKS], F32, tag="pacc", name="pacc")
        epsb = consts.tile([128, 1], F32, tag="epsb", name="epsb")
        zb = consts.tile([128, 1], F32, tag="zb", name="zb")
        nc.vector.memset(epsb[:], BN_EPS)
        nc.vector.memset(zb[:], 0.0)

        # ---- collective warmup ------------------------------------------
        warm_in = dram.tile([128, 1], F32, tag="wi", name="wi")
        warm_out = dram.tile([128, 1], F32, tag="wo", name="wo",
                             addr_space="Shared")
        nc.sync.dma_start(out=warm_in[:], in_=zb[:])
        nc.gpsimd.collective_compute(
            "AllReduce", ALU.add, replica_groups=RG,
            ins=[warm_in[:].opt()], outs=[warm_out[:].opt()],
        )

        # ---- input DMAs (sync queue, dependency-ordered) ----------------
        piece = []
        for ci in range(NCHUNKS):
            lo = 0 if ci == 0 else GUARD + ci * CHUNK
            hi = XBUF if ci == NCHUNKS - 1 else GUARD + (ci + 1) * CHUNK
            piece.append((lo, hi))

        def xpiece(ci):
            lo, hi = piece[ci]
            for ct in range(CT):
                nc.sync.dma_start(out=xb3[1][:, ct, lo:hi], in_=xbp[ct, :, lo:hi])

        def shcopy(ci):
            lo = GUARD + ci * CHUNK
            for d, off in ((0, -1), (2, 1)):
                nc.sync.dma_start(
                    out=xb3[d][:, :, lo : lo + CHUNK],
                    in_=xb3[1][:, :, lo + off : lo + CHUNK + off],
                )

        WFS = MT1 * 128 // 3
        xpiece(0)
        for kt in range(CT):
            nc.sync.dma_start(out=wf_sb[kt][:, 0:WFS], in_=wf[kt, :, 0:WFS])
        nc.sync.dma_start(out=bfp_sb[:], in_=bfp[:, :])
        nc.sync.dma_start(out=wecab[:], in_=weca[0:1, :].to_broadcast([128, 3]))
        xpiece(1)
        shcopy(0)
        for kt in range(CT):
            nc.sync.dma_start(
                out=wf_sb[kt][:, WFS : 2 * WFS], in_=wf[kt, :, WFS : 2 * WFS]
            )
        xpiece(2)
        shcopy(1)
        for kt in range(CT):
            nc.sync.dma_start(
                out=wf_sb[kt][:, 2 * WFS :], in_=wf[kt, :, 2 * WFS :]
            )
        xpiece(3)
        shcopy(2)
        shcopy(3)
        for kt in range(CT):
            nc.sync.dma_start(out=wp_sb[kt][:], in_=wp[kt])
        nc.sync.dma_start(out=gam_sb[:], in_=gam[:, :])
        nc.sync.dma_start(out=bet_sb[:], in_=bet[:, :])

        # guard zeros for the derived buffers (vector, head only)
        for d in (0, 2):
            nc.vector.memset(xb3[d][:, :, 0:GUARD], 0.0)
            nc.vector.memset(xb3[d][:, :, GUARD + HW : XBUF], 0.0)

        # wrapped-column fixes for the derived buffers (gpsimd)
        def wrapfix(ci):
            lo = GUARD + ci * CHUNK
            v0 = xb3[0][:, :, lo : lo + CHUNK].rearrange(
                "p c (r w) -> p c r w", w=W)
            nc.gpsimd.memset(v0[:, :, :, 0:1], 0.0)
            v2 = xb3[2][:, :, lo : lo + CHUNK].rearrange(
                "p c (r w) -> p c r w", w=W)
            nc.gpsimd.memset(v2[:, :, :, W - 1 : W], 0.0)

        for ci in range(NCHUNKS):
            wrapfix(ci)

        # ---- ECA pooling (scalar accum) + combine (gpsimd) --------------
        def pool_piece(ci):
            lo = GUARD + ci * CHUNK
            for ct in range(CT):
                nc.scalar.activation(
                    out=pscr[:], in_=xb3[1][:, ct, lo : lo + CHUNK],
                    func=AF.Copy, accum_out=pacc[:, ct, ci : ci + 1],
                )

        pool2 = consts.tile([128, CT], F32, tag="pool2", name="pool2")
        shd = consts.tile([128, CT], F32, tag="shd", name="shd")
        shu = consts.tile([128, CT], F32, tag="shu", name="shu")
        eca1 = consts.tile([128, CT], F32, tag="eca1", name="eca1")
        eca2 = consts.tile([128, CT], F32, tag="eca2", name="eca2")
        attn = consts.tile([128, CT], F32, tag="attn", name="attn")

        def emit_eca_combine():
            # pool2 = sum over the 4 chunk partials (gpsimd, tiny)
            nc.gpsimd.tensor_tensor(
                out=pool2[:], in0=pacc[:, :, 0], in1=pacc[:, :, 1], op=ALU.add
            )
            nc.gpsimd.tensor_tensor(
                out=pool2[:], in0=pool2[:], in1=pacc[:, :, 2], op=ALU.add
            )
            nc.gpsimd.tensor_tensor(
                out=pool2[:], in0=pool2[:], in1=pacc[:, :, 3], op=ALU.add
            )
            nc.gpsimd.memset(shd[:], 0.0)
            nc.gpsimd.memset(shu[:], 0.0)
            for ct in range(CT):
                nc.gpsimd.dma_start(
                    out=shd[1:128, ct : ct + 1], in_=pool2[0:127, ct : ct + 1]
                )
                nc.gpsimd.dma_start(
                    out=shu[0:127, ct : ct + 1], in_=pool2[1:128, ct : ct + 1]
                )
            nc.gpsimd.dma_start(out=shd[0:1, 1:2], in_=pool2[127:128, 0:1])
            nc.gpsimd.dma_start(out=shu[127:128, 0:1], in_=pool2[0:1, 1:2])
            nc.vector.tensor_scalar(
                out=eca1, in0=shd[:], scalar1=wecab[:, 0:1], scalar2=None,
                op0=ALU.mult,
            )
            nc.vector.scalar_tensor_tensor(
                out=eca2, in0=pool2[:], scalar=wecab[:, 1:2], in1=eca1[:],
                op0=ALU.mult, op1=ALU.add,
            )
            nc.vector.scalar_tensor_tensor(
                out=eca1, in0=shu[:], scalar=wecab[:, 2:3], in1=eca2[:],
                op0=ALU.mult, op1=ALU.add,
            )

        # ---- main loop ---------------------------------------------------
        fused_t = [None] * NCHUNKS
        p7_t = [None] * NCHUNKS
        p8_t = [None] * NCHUNKS
        ypt_t = [None] * NCHUNKS
        coff = [GUARD + ci * CHUNK for ci in range(NCHUNKS)]

        def emit_mm1_chunk(ci, scalar_hooks=None):
            r0 = ci * ROWS_PER_CHUNK
            prods = []
            for k in range(KS * KS):
                di, dj = divmod(k, KS)
                woff = GUARD + (r0 + di - 1) * W
                nbufs = 2 if k >= 7 else 1
                pr = prod_pool.tile([128, CT, CHUNK], BF16, tag=f"pr{k}",
                                    name=f"pr{k}", bufs=nbufs)
                fsb = fsb_pool.tile([128, CT, CHUNK], BF16, tag="fsb",
                                    name="fsb")
                for ct in range(CT):
                    mt = k * CT + ct
                    fp = fps.tile([128, CHUNK], F32, tag="fp", name="fp")
                    for kt in range(CT):
                        lhsT = wf_sb[kt][:, mt * 128 : (mt + 1) * 128]
                        for nh in range(NH):
                            rhs = xb3[1][:, kt,
                                         coff[ci] + nh * 512 : coff[ci] + (nh + 1) * 512]
                            nc.tensor.matmul(
                                fp[:, nh * 512 : (nh + 1) * 512],
                                lhsT,
                                rhs,
                                start=(kt == 0),
                                stop=(kt == CT - 1),
                            )
                    nc.scalar.activation(
                        out=fsb[:, ct, :], in_=fp[:], func=AF.Identity,
                        bias=bfp_sb[:, mt : mt + 1], scale=1.0,
                    )
                if scalar_hooks and k in scalar_hooks:
                    scalar_hooks[k]()
                # tap product against the shifted window (both ct at once)
                nc.vector.tensor_tensor(
                    out=pr[:],
                    in0=fsb[:],
                    in1=xb3[dj][:, :, woff : woff + CHUNK],
                    op=ALU.mult,
                )
                prods.append(pr)
                # weave the add tree
                if k == 1:
                    nc.vector.tensor_add(prods[0][:], prods[0][:], prods[1][:])
                elif k == 3:
                    nc.vector.tensor_add(prods[2][:], prods[2][:], prods[3][:])
                    nc.vector.tensor_add(prods[0][:], prods[0][:], prods[2][:])
                elif k == 5:
                    nc.vector.tensor_add(prods[4][:], prods[4][:], prods[5][:])
                elif k == 6:
                    nc.vector.tensor_add(prods[4][:], prods[4][:], prods[6][:])
                    ft = prod_pool.tile([128, CT, CHUNK], BF16, tag="fused",
                                        name="fused", bufs=2)
                    nc.vector.tensor_add(ft[:], prods[0][:], prods[4][:])
                    fused_t[ci] = ft
            p7_t[ci] = prods[7]
            p8_t[ci] = prods[8]

        def emit_mm2_chunk(ci):
            # srcs: fused, p7, p8 against wp; center x against weff (last so
            # the attn-dependent weights have maximal slack)
            ypt = []
            for mt2 in range(CT):
                yp = [yps.tile([128, 512], F32, tag="yp", name="yp")
                      for _ in range(NH)]
                srcs = [(fused_t[ci], wp_sb), (p7_t[ci], wp_sb),
                        (p8_t[ci], wp_sb), (None, weff)]
                ns = len(srcs)
                for si, (srct, wtab) in enumerate(srcs):
                    for kt in range(CT):
                        lhsT2 = wtab[kt][:, mt2 * 128 : (mt2 + 1) * 128]
                        for nh in range(NH):
                            if srct is None:
                                rhs = xb3[1][:, kt,
                                             coff[ci] + nh * 512 : coff[ci] + (nh + 1) * 512]
                            else:
                                rhs = srct[:, kt, nh * 512 : (nh + 1) * 512]
                            nc.tensor.matmul(
                                yp[nh][:],
                                lhsT2,
                                rhs,
                                start=(si == 0 and kt == 0),
                                stop=(si == ns - 1 and kt == CT - 1),
                            )
                ypt.append(yp)
            ypt_t[ci] = ypt

        def emit_yev(ci):
            # mm2 PSUM -> y_sb (scalar engine)
            r0 = ci * ROWS_PER_CHUNK
            for mt2 in range(CT):
                for nh in range(NH):
                    src = ypt_t[ci][mt2][nh]
                    dst = y_sb[mt2][:, r0 * W + nh * 512 : r0 * W + (nh + 1) * 512]
                    nc.scalar.activation(out=dst, in_=src[:], func=AF.Copy)

        def emit_bn(ci):
            for mt2 in range(CT):
                for nh in range(NH):
                    nc.vector.bn_stats(
                        out=stats_sb[mt2][:, ci * NH + nh, :],
                        in_=ypt_t[ci][mt2][nh][:],
                    )

        def emit_weff():
            for kt in range(CT):
                nc.vector.tensor_scalar(
                    out=weff[kt][:], in0=wp_sb[kt][:],
                    scalar1=attn[:, kt : kt + 1], scalar2=None, op0=ALU.mult,
                )

        pending = None
        for ci in range(NCHUNKS):
            if pending is not None:
                emit_yev(pending)
                emit_bn(pending)
                pending = None
            if ci == 0:
                hooks = {
                    0: lambda: pool_piece(0),
                    2: lambda: pool_piece(1),
                    4: lambda: pool_piece(2),
                    6: lambda: pool_piece(3),
                }
                emit_mm1_chunk(0, scalar_hooks=hooks)
                emit_eca_combine()
                nc.scalar.activation(out=attn[:], in_=eca1[:],
                                     func=AF.Sigmoid, bias=zb[:, 0:1])
                emit_weff()
            else:
                emit_mm1_chunk(ci)
                emit_mm2_chunk(ci - 1)
                pending = ci - 1

        emit_mm2_chunk(NCHUNKS - 1)
        emit_yev(NCHUNKS - 2)
        emit_bn(NCHUNKS - 2)
        emit_yev(NCHUNKS - 1)
        emit_bn(NCHUNKS - 1)

        # ---- global BN stats via all-reduce -----------------------------
        ps = consts.tile([128, CT, 2], F32, tag="ps", name="ps")
        for mt2 in range(CT):
            mv = consts.tile([128, 2], F32, tag=f"mv{mt2}", name=f"mv{mt2}")
            nc.vector.bn_aggr(out=mv[:], in_=stats_sb[mt2][:])
            nc.vector.tensor_scalar(
                out=ps[:, mt2, 0:1], in0=mv[:, 0:1], scalar1=1.0, scalar2=None,
                op0=ALU.mult,
            )
            nc.vector.scalar_tensor_tensor(
                out=ps[:, mt2, 1:2], in0=mv[:, 0:1], scalar=mv[:, 0:1],
                in1=mv[:, 1:2], op0=ALU.mult, op1=ALU.add,
            )
        nc.vector.tensor_scalar(
            out=ps[:], in0=ps[:], scalar1=float(HW), scalar2=None, op0=ALU.mult
        )

        ps_b = dram.tile([128, CT * 2], F32, tag="psb", name="psb")
        gs_b = dram.tile([128, CT * 2], F32, tag="gsb", name="gsb",
                         addr_space="Shared")
        nc.scalar.dma_start(out=ps_b[:], in_=ps.rearrange("p m two -> p (m two)"))
        nc.gpsimd.collective_compute(
            "AllReduce", ALU.add, replica_groups=RG,
            ins=[ps_b[:].opt()], outs=[gs_b[:].opt()],
        )
        gs = consts.tile([128, CT, 2], F32, tag="gs", name="gs")
        nc.sync.dma_start(out=gs.rearrange("p m two -> p (m two)"), in_=gs_b[:])

        # ---- normalize and write out ------------------------------------
        minv = 1.0 / float(B * HW)
        mg = consts.tile([128, CT], F32, tag="mg", name="mg")
        vg = consts.tile([128, CT], F32, tag="vg", name="vg")
        rr = consts.tile([128, CT], F32, tag="rr", name="rr")
        tt = consts.tile([128, CT], F32, tag="tt", name="tt")
        ac = consts.tile([128, CT], F32, tag="ac", name="ac")
        bc = consts.tile([128, CT], F32, tag="bc", name="bc")
        nc.vector.tensor_scalar(
            out=mg[:], in0=gs[:, :, 0], scalar1=minv, scalar2=None, op0=ALU.mult
        )
        nc.vector.tensor_scalar(
            out=vg[:], in0=gs[:, :, 1], scalar1=minv, scalar2=None, op0=ALU.mult
        )
        nc.vector.tensor_tensor(out=tt[:], in0=mg[:], in1=mg[:], op=ALU.mult)
        nc.vector.tensor_tensor(out=vg[:], in0=vg[:], in1=tt[:], op=ALU.subtract)
        nc.scalar.activation(out=tt[:], in_=vg[:], func=AF.Sqrt,
                             bias=epsb[:, 0:1], scale=1.0)
        nc.vector.reciprocal(out=rr[:], in_=tt[:])
        nc.vector.tensor_tensor(out=ac[:], in0=rr[:], in1=gam_sb[:], op=ALU.mult)
        nc.vector.tensor_tensor(out=bc[:], in0=mg[:], in1=ac[:], op=ALU.mult)
        nc.vector.tensor_tensor(out=bc[:], in0=bet_sb[:], in1=bc[:], op=ALU.subtract)

        NSL = 4
        SL = HW // NSL
        for s in range(NSL):
            for mt2 in range(CT):
                sl = slice(s * SL, (s + 1) * SL)
                st = stage_pool.tile([128, SL], BF16, tag="st", name="st")
                if (s * CT + mt2) % 8 < 5:
                    nc.vector.tensor_scalar(
                        out=st[:], in0=y_sb[mt2][:, sl],
                        scalar1=ac[:, mt2 : mt2 + 1], scalar2=bc[:, mt2 : mt2 + 1],
                        op0=ALU.mult, op1=ALU.add,
                    )
                else:
                    nc.scalar.activation(
                        out=st[:], in_=y_sb[mt2][:, sl],
                        func=AF.Identity, bias=bc[:, mt2 : mt2 + 1],
                        scale=ac[:, mt2 : mt2 + 1],
                    )
                nc.sync.dma_start(out=yout[mt2, :, sl], in_=st[:])


_NC = None


def _build_nc(debug=False):
    nc = bacc.Bacc(
        "TRN2", target_bir_lowering=False, debug=debug, num_devices=NCORES
    )
    with tile.TileContext(nc, num_cores=NCORES) as tc:
        _emit(tc)
    nc.compile()
    return nc


def _get_nc():
    global _NC
    if _NC is None:
        _NC = _build_nc()
    return _NC


def _prep_in_maps(x, W_filter, b_filter, w_eca, W_proj, gamma, beta):
    bf = ml_dtypes.bfloat16
    x = np.asarray(x, np.float32)
    W_filter = np.asarray(W_filter, np.float32)
    b_filter = np.asarray(b_filter, np.float32)
    w_eca = np.asarray(w_eca, np.float32)
    W_proj = np.asarray(W_proj, np.float32)
    gamma = np.asarray(gamma, np.float32)
    beta = np.asarray(beta, np.float32)

    # center window buffer: [64 zeros][x flat][64 zeros]
    buf = np.zeros((B, C, XBUF), np.float32)
    buf[:, :, GUARD : GUARD + HW] = x.reshape(B, C, HW)
    xb_h = np.ascontiguousarray(buf.reshape(B, CT, 128, XBUF)).astype(bf)

    # permute mm1 weights: o' = k*256 + c  (original o = c*9 + k)
    wperm = W_filter.reshape(C, KS * KS, C).transpose(1, 0, 2).reshape(KS * KS * C, C)
    wf_h = np.ascontiguousarray(wperm.T.reshape(CT, 128, MT1 * 128)).astype(bf)
    bperm = b_filter.reshape(C, KS * KS).T.reshape(KS * KS * C)
    bfp_h = np.ascontiguousarray(bperm.reshape(MT1, 128).T).astype(np.float32)

    wp_h = np.ascontiguousarray((0.5 * W_proj).T.reshape(CT, 128, C)).astype(bf)
    weca_h = (w_eca / float(HW)).reshape(1, 3).astype(np.float32)
    gam_h = np.ascontiguousarray(gamma.reshape(CT, 128).T).astype(np.float32)
    bet_h = np.ascontiguousarray(beta.reshape(CT, 128).T).astype(np.float32)

    in_maps = []
    for i in range(B):
        m = {
            "xb": xb_h[i],
            "wf": wf_h,
            "bfp": bfp_h,
            "wp": wp_h,
            "weca": weca_h,
            "gam": gam_h,
            "bet": bet_h,
        }
        in_maps.append(m)
    return in_maps


last_result = None


def kernel(x, W_filter, b_filter, w_eca, W_proj, b_proj, gamma, beta):
    """Full-input, full-output DDF module on 8 NeuronCores."""
    global last_result
    # b_proj is mathematically cancelled by the batch-norm; unused.
    in_maps = _prep_in_maps(x, W_filter, b_filter, w_eca, W_proj, gamma, beta)
    nc = _get_nc()
    trace = bool(int(os.environ.get("DDF_TRACE", "0")))
    res = run_bass_kernel_spmd(nc, in_maps, list(range(NCORES)), trace=trace)
    last_result = res
    out = np.stack(
        [
            np.asarray(res.results[i]["y"]).reshape(C, H, W).astype(np.float32)
            for i in range(B)
        ]
    )
    return out
